# revision 1
# baseline (speedup 1.0000x reference)
"""GQA attention layer (B=2, S=2048, D=4096, 32 Q heads / 8 KV heads, RoPE,
causal) on 8 Trainium2 NeuronCores, tensor-parallel over heads.

Each core owns 4 Q heads + 1 KV head: it computes its Q/K/V projections,
RoPE, causal attention, and a partial output projection (rank-512 slice of
the wo contraction).  The host sums the 8 partial outputs.

Layouts are feature-major ("transposed") on chip: activations live as
[feature_partition, token_free] so every matmul contracts over the
partition dim with wide (>=256) moving operands, keeping the PE at full
rate with float32r (fp22-precision fp32) operands.
"""

import os
import sys
import types
from contextlib import ExitStack

import numpy as np

import concourse.bass as bass
import concourse.tile as tile
from concourse import bacc
from concourse import mybir
from concourse import bass_utils
from concourse.bass_utils import run_bass_kernel_spmd

# ---------------------------------------------------------------------------
# Optional NTFF profiling support under axon. The trimmed image's `antenv`
# lacks `axon_hooks`, so run_bass_kernel_spmd(trace=True) would silently skip
# tracing; register the hook ourselves. Harmless when unavailable.
try:
    import antenv  # noqa: F401
    from trn_agent_boot.trn_boot import _ntff_profile_via_ctypes

    if "antenv.axon_hooks" not in sys.modules:
        _hooks_mod = types.ModuleType("antenv.axon_hooks")
        _hook = _ntff_profile_via_ctypes("/opt/axon/libaxon_pjrt.so")
        _hooks_mod.get_axon_ntff_profile_hook = lambda: _hook
        _hooks_mod.set_axon_ntff_profile_hook = lambda h: None
        sys.modules["antenv.axon_hooks"] = _hooks_mod
    bass_utils.upload_artifacts = lambda tmpdir: "local://skipped"
except Exception:
    pass

F32 = mybir.dt.float32
F32R = mybir.dt.float32r
EXP = mybir.ActivationFunctionType.Exp

B, S, D = 2, 2048, 4096
NH, NKV, HD = 32, 8, 128
T = B * S                       # 4096 tokens total
N_CORES = 8
QH = NH // N_CORES              # 4 local q heads
FL = QH * HD                    # 512 local q features
SCALE = 1.0 / float(np.sqrt(HD))
NEG = -1.0e30

NW = 512                        # token-group width in the QKV projection
QT = 256                        # q-token group width in attention (AV moving dim)
DKD = D // 128                  # 32 contraction chunks for projections


def _build_program():
    nc = bacc.Bacc("TRN2", target_bir_lowering=False, debug=False,
                   num_devices=N_CORES)

    xT = nc.dram_tensor("xT", [D, T], F32R, kind="ExternalInput").ap()
    wqT = nc.dram_tensor("wqT", [D, FL], F32R, kind="ExternalInput").ap()
    wkT = nc.dram_tensor("wkT", [D, HD], F32R, kind="ExternalInput").ap()
    wvT = nc.dram_tensor("wvT", [D, HD], F32R, kind="ExternalInput").ap()
    woT = nc.dram_tensor("woT", [FL, D], F32R, kind="ExternalInput").ap()
    # RoPE constants, pre-assembled for the rotate-half formulation on the
    # even/odd-split feature layout: ropc = [cos; cos], rops = [-sin; sin].
    ropc = nc.dram_tensor("ropc", [HD, S], F32, kind="ExternalInput").ap()
    rops = nc.dram_tensor("rops", [HD, S], F32, kind="ExternalInput").ap()
    idin = nc.dram_tensor("idin", [128, 128], F32R, kind="ExternalInput").ap()
    onesin = nc.dram_tensor("onesin", [128, 1], F32R, kind="ExternalInput").ap()
    maskt = [nc.dram_tensor(f"maskt{v}", [128, 4 * 128], F32,
                            kind="ExternalInput").ap() for v in range(4)]
    y = nc.dram_tensor("y", [T, D], F32, kind="ExternalOutput").ap()

    with tile.TileContext(nc) as tc, ExitStack() as ctx:
        dram = ctx.enter_context(tc.tile_pool(name="dram", bufs=1, space="DRAM"))
        qT_d = [dram.tile([FL, S], F32R, tag=f"qT_d{b}", name=f"qT_d{b}")
                for b in range(B)]
        kT_d = [dram.tile([HD, S], F32R, tag=f"kT_d{b}", name=f"kT_d{b}")
                for b in range(B)]
        vT_d = [dram.tile([HD, S], F32R, tag=f"vT_d{b}", name=f"vT_d{b}")
                for b in range(B)]

        const = ctx.enter_context(tc.tile_pool(name="const", bufs=1))
        ident = const.tile([128, 128], F32R)
        nc.sync.dma_start(ident[:], idin)
        ones_t = const.tile([128, 1], F32R)
        nc.sync.dma_start(ones_t[:], onesin)
        mtv = []
        for v in range(4):
            mt = const.tile([128, 4 * 128], F32, tag=f"mtv{v}", name=f"mtv{v}")
            nc.sync.dma_start(mt[:], maskt[v])
            mtv.append(mt)

        # ------------------------------------------------------------------
        # Phase 1: QKV projections + RoPE  ->  DRAM scratch (feature-major)
        # ------------------------------------------------------------------
        with tc.tile_pool(name="wqkv", bufs=1) as wpool, \
             tc.tile_pool(name="ropec", bufs=1) as rcpool, \
             tc.tile_pool(name="xin", bufs=3) as xpool, \
             tc.tile_pool(name="qkvstage", bufs=2) as stage, \
             tc.tile_pool(name="ropetmp", bufs=2) as rtmp, \
             tc.tile_pool(name="qkvps", bufs=1, space="PSUM") as qkvps:

            cos_s = rcpool.tile([HD, S], F32)
            nc.sync.dma_start(cos_s[:], ropc)
            sin_s = rcpool.tile([HD, S], F32)
            nc.sync.dma_start(sin_s[:], rops)

            # Resident weights, packed k-chunk-major: [128, DKD * width]
            wq_sb = wpool.tile([128, DKD * FL], F32R, tag="wq")
            nc.sync.dma_start(
                wq_sb[:].rearrange("p (k f) -> p k f", k=DKD),
                wqT.rearrange("(k p) f -> p k f", p=128))
            wk_sb = wpool.tile([128, DKD * HD], F32R, tag="wk")
            nc.sync.dma_start(
                wk_sb[:].rearrange("p (k f) -> p k f", k=DKD),
                wkT.rearrange("(k p) f -> p k f", p=128))
            wv_sb = wpool.tile([128, DKD * HD], F32R, tag="wv")
            nc.sync.dma_start(
                wv_sb[:].rearrange("p (k f) -> p k f", k=DKD),
                wvT.rearrange("(k p) f -> p k f", p=128))

            def rope_evict(ps, out_sb, pos0, use_dve=False):
                """out_sb = RoPE(ps) on the even/odd-split feature layout
                (partitions 0..63 even pair components, 64..127 odd):
                out = x * [c;c] + swap_halves(x) * [-s;s]."""
                c = cos_s[:, pos0:pos0 + NW]
                s = sin_s[:, pos0:pos0 + NW]
                xsb = rtmp.tile([128, NW], F32, tag="xsb")
                if use_dve:
                    nc.vector.tensor_copy(xsb[:], ps[:])
                else:
                    nc.scalar.copy(xsb[:], ps[:])
                xsw = rtmp.tile([128, NW], F32, tag="xsw")
                nc.sync.dma_start(xsw[0:64, :], xsb[64:128, :])
                nc.sync.dma_start(xsw[64:128, :], xsb[0:64, :])
                t1 = rtmp.tile([128, NW], F32, tag="t1")
                nc.vector.tensor_mul(t1[:], xsw[:], s)
                nc.vector.tensor_mul(out_sb[:], xsb[:], c)
                nc.vector.tensor_add(out_sb[:], out_sb[:], t1[:])

            for n in range(T // NW):
                pos0 = (n * NW) % S
                qps = [qkvps.tile([128, NW], F32, tag=f"qps{m}", name=f"qps{m}")
                       for m in range(QH)]
                kps = qkvps.tile([128, NW], F32, tag="kps")
                vps = qkvps.tile([128, NW], F32, tag="vps")
                for k in range(DKD):
                    xt = xpool.tile([128, NW], F32R)
                    nc.sync.dma_start(
                        xt[:], xT[k * 128:(k + 1) * 128, n * NW:(n + 1) * NW])
                    st = (k == 0)
                    sp = (k == DKD - 1)
                    for m in range(QH):
                        nc.tensor.matmul(
                            qps[m][:],
                            wq_sb[:, k * FL + m * 128:k * FL + (m + 1) * 128],
                            xt[:], start=st, stop=sp)
                    nc.tensor.matmul(
                        kps[:], wk_sb[:, k * HD:(k + 1) * HD], xt[:],
                        start=st, stop=sp)
                    nc.tensor.matmul(
                        vps[:], wv_sb[:, k * HD:(k + 1) * HD], xt[:],
                        start=st, stop=sp)
                for m in range(QH):
                    qst = stage.tile([128, NW], F32R, tag=f"qst{m}", name=f"qst{m}")
                    rope_evict(qps[m], qst, pos0, use_dve=(m % 2 == 1))
                    nc.sync.dma_start(
                        qT_d[n * NW // S][m * 128:(m + 1) * 128,
                                          (n * NW) % S:(n * NW) % S + NW], qst[:])
                kst = stage.tile([128, NW], F32R, tag="kst")
                rope_evict(kps, kst, pos0)
                nc.sync.dma_start(
                    kT_d[n * NW // S][:, (n * NW) % S:(n * NW) % S + NW], kst[:])
                vst = stage.tile([128, NW], F32R, tag="vst")
                nc.vector.tensor_copy(vst[:], vps[:])
                nc.sync.dma_start(
                    vT_d[n * NW // S][:, (n * NW) % S:(n * NW) % S + NW], vst[:])

        # ------------------------------------------------------------------
        # Phase 2: attention + output projection
        # ------------------------------------------------------------------
        with tc.tile_pool(name="wo", bufs=1) as wopool, \
             tc.tile_pool(name="kv", bufs=2) as kvpool, \
             tc.tile_pool(name="qheads", bufs=1) as qpool, \
             tc.tile_pool(name="ptiles", bufs=4) as ptpool, \
             tc.tile_pool(name="attn", bufs=2) as atpool, \
             tc.tile_pool(name="smax", bufs=2) as smpool, \
             tc.tile_pool(name="ystage", bufs=2) as ypool, \
             tc.tile_pool(name="sps", bufs=2, space="PSUM") as spsum, \
             tc.tile_pool(name="vtps", bufs=1, space="PSUM") as vtpsum, \
             tc.tile_pool(name="sums", bufs=1, space="PSUM") as smpsum, \
             tc.tile_pool(name="avps", bufs=2, space="PSUM") as avpsum, \
             tc.tile_pool(name="yps", bufs=2, space="PSUM") as ypsum:

            QB = 512                       # q-block width in attention
            wo_sb = wopool.tile([128, QH * D], F32R)
            nc.sync.dma_start(
                wo_sb[:].rearrange("p (f d) -> p f d", f=QH),
                woT.rearrange("(f p) d -> p f d", p=128))

            for b in range(B):
                t0 = b * S
                ktb = kvpool.tile([128, S], F32R, tag="ktb")
                nc.sync.dma_start(ktb[:], kT_d[b][:, :])
                # V token-major: V_b[:, kc*128:+128] = vT[:, kc-block].T
                vtb = kvpool.tile([128, S], F32R, tag="vtb")
                nc.sync.dma_start(vtb[:], vT_d[b][:, :])
                V_b = kvpool.tile([128, S], F32R, tag="V_b")
                for kc in range(S // 128):
                    vt_ps = vtpsum.tile([128, 128], F32R)
                    nc.tensor.transpose(
                        vt_ps[:], vtb[:, kc * 128:(kc + 1) * 128], ident[:])
                    nc.vector.tensor_copy(
                        V_b[:, kc * 128:(kc + 1) * 128], vt_ps[:])

                qtb = [qpool.tile([128, S], F32R, tag=f"qtb{h}", name=f"qtb{h}")
                       for h in range(QH)]
                for h in range(QH):
                    nc.sync.dma_start(
                        qtb[h][:], qT_d[b][h * 128:(h + 1) * 128, :])

                def emit_wo(att_prev, q0_prev):
                    for tcx in range(QB // 128):
                        tg0 = t0 + q0_prev + tcx * 128
                        for dg in range(D // NW):
                            yp = ypsum.tile([128, NW], F32)
                            for f in range(QH):
                                nc.tensor.matmul(
                                    yp[:],
                                    att_prev[f][:, tcx * 128:(tcx + 1) * 128],
                                    wo_sb[:, f * D + dg * NW:f * D + (dg + 1) * NW],
                                    start=(f == 0), stop=(f == QH - 1))
                            ysb = ypool.tile([128, NW], F32)
                            nc.scalar.copy(ysb[:], yp[:])
                            nc.sync.dma_start(
                                y[tg0:tg0 + 128, dg * NW:(dg + 1) * NW], ysb[:])

                pending = None
                for qb in range(S // QB):
                    nkt = (qb + 1) * (QB // 128)     # causal 128-wide kt chunks
                    q0 = qb * QB
                    att = [atpool.tile([128, QB], F32R, tag=f"att{h}",
                                       name=f"att{h}") for h in range(QH)]
                    for h in range(QH):
                        # S.T = k.T-stationary @ q-moving: [kt, q]; exp
                        # straight from PSUM (no max subtraction: |scale*S|
                        # is small); causal mask added on the diagonal
                        # chunks; row sums via a ones-column matmul.
                        avp = avpsum.tile([128, QB], F32)
                        smp = smpsum.tile([1, QB], F32)
                        for ktc in range(nkt):
                            stp = spsum.tile([128, QB], F32)
                            nc.tensor.matmul(
                                stp[:], ktb[:, ktc * 128:(ktc + 1) * 128],
                                qtb[h][:, q0:q0 + QB], start=True, stop=True)
                            if ktc >= nkt - 4:
                                nc.vector.tensor_add(
                                    stp[:], stp[:], mtv[ktc - (nkt - 4)][:])
                            pt = ptpool.tile([128, QB], F32R)
                            nc.scalar.activation(pt[:], stp[:], EXP, scale=SCALE)
                            nc.tensor.matmul(
                                avp[:], V_b[:, ktc * 128:(ktc + 1) * 128],
                                pt[:], start=(ktc == 0), stop=(ktc == nkt - 1))
                            nc.tensor.matmul(
                                smp[:], ones_t[:], pt[:],
                                start=(ktc == 0), stop=(ktc == nkt - 1))
                        # Fast PSUM eviction (ACT copies), then normalize off
                        # the critical path: att = att_un * (1/sums).
                        attu = atpool.tile([128, QB], F32, tag=f"attu{h}",
                                           name=f"attu{h}", bufs=1)
                        nc.scalar.copy(attu[:], avp[:])
                        s_sb = smpool.tile([1, QB], F32, tag="s_sb")
                        nc.scalar.copy(s_sb[:], smp[:])
                        r_sb = smpool.tile([1, QB], F32, tag="r_sb")
                        nc.vector.reciprocal(r_sb[:], s_sb[:])
                        r_bc = smpool.tile([128, QB], F32, tag="r_bc")
                        nc.gpsimd.partition_broadcast(r_bc[:], r_sb[:])
                        nc.vector.tensor_mul(att[h][:], attu[:], r_bc[:])
                    # previous q block's output projection, emitted here so
                    # its PE work queues behind this block's attention and
                    # never stalls the in-order PE stream on normalization
                    if pending is not None:
                        emit_wo(*pending)
                    pending = (att, q0)
                if pending is not None:
                    emit_wo(*pending)
    nc.compile()
    return nc


_program = None


def _get_program():
    global _program
    if _program is None:
        _program = _build_program()
    return _program


def kernel(**inputs) -> np.ndarray:
    x = np.asarray(inputs["x"], dtype=np.float32)
    wq = np.asarray(inputs["wq"], dtype=np.float32)
    wk = np.asarray(inputs["wk"], dtype=np.float32)
    wv = np.asarray(inputs["wv"], dtype=np.float32)
    wo = np.asarray(inputs["wo"], dtype=np.float32)
    cos = np.asarray(inputs["freqs_cos"], dtype=np.float32)
    sin = np.asarray(inputs["freqs_sin"], dtype=np.float32)
    mask = np.asarray(inputs["mask"], dtype=np.float32)
    start_pos = int(np.asarray(inputs.get("start_pos", 0)))
    assert start_pos == 0, "kernel specialized for start_pos == 0"

    # Even/odd RoPE pair split within each head's 128 features.
    perm = np.concatenate([np.arange(0, HD, 2), np.arange(1, HD, 2)])

    xT = np.ascontiguousarray(x.reshape(T, D).T)
    cosT = cos.T                                   # [64, S]
    sinT = sin.T
    ropc = np.ascontiguousarray(np.concatenate([cosT, cosT], axis=0))
    rops = np.ascontiguousarray(np.concatenate([-sinT, sinT], axis=0))
    # Transposed diagonal-mask variants: for kt chunk at offset v*128 within
    # a 512-token q block, maskt_v[r, c] = clamp(mask[c, v*128 + r]).
    masktv = [np.ascontiguousarray(
        np.maximum(mask[:512, v * 128:(v + 1) * 128], NEG).astype(np.float32).T)
        for v in range(4)]

    in_maps = []
    for c in range(N_CORES):
        wq_c = wq[c * FL:(c + 1) * FL].reshape(QH, HD, D)[:, perm, :].reshape(FL, D)
        wk_c = wk[c * HD:(c + 1) * HD][perm, :]
        wv_c = wv[c * HD:(c + 1) * HD]
        wo_c = wo[:, c * FL:(c + 1) * FL]
        in_maps.append({
            "xT": xT,
            "idin": np.eye(128, dtype=np.float32),
            "wqT": np.ascontiguousarray(wq_c.T),
            "wkT": np.ascontiguousarray(wk_c.T),
            "wvT": np.ascontiguousarray(wv_c.T),
            "woT": np.ascontiguousarray(wo_c.T),
            "ropc": ropc,
            "rops": rops,
            "onesin": np.ones((128, 1), dtype=np.float32),
            "maskt0": masktv[0],
            "maskt1": masktv[1],
            "maskt2": masktv[2],
            "maskt3": masktv[3],
        })

    nc = _get_program()
    trace = bool(int(os.environ.get("GQA_TRACE", "0")))
    kwargs = {}
    if trace:
        tmpdir = os.environ.get("GQA_TRACE_DIR") or None
        kwargs = dict(trace=True, tmpdir=tmpdir, trace_cores=[0])
    res = run_bass_kernel_spmd(nc, in_maps, list(range(N_CORES)), **kwargs)
    kernel.last_results = res

    acc = np.zeros((T, D), dtype=np.float64)
    for c in range(N_CORES):
        acc += res.results[c]["y"]
    return acc.astype(np.float32).reshape(B, S, D)

